# revision 8
# baseline (speedup 1.0000x reference)
"""MoE feed-forward (E=8 experts, top-2) for one TRN2 chip (8 NeuronCores).

Strategy: expert-parallel. Host computes the (tiny) router matmul + softmax
+ top-2 in numpy, gathers each expert's routed tokens, pads to a fixed
capacity C, and ships per-expert weights + gathered tokens to one core each.
Each core runs an identical Bass/Tile FFN program in bf16:

    GT = Wg^T @ X   (transposed-activation layout: [I, C] tiles)
    UT = Wu^T @ X
    AT = silu(GT) * UT          (bf16, SBUF-resident)
    YT = Wd^T_col-tiles @ AT    -> [H, C] f32 out

All matmul operands use natural (row-major chunk) layouts, so no on-device
transposes are needed. Gate/up weights are DMA'd in per-i-tile column blocks
so arrival order matches the PE's consumption order (the first i-tile needs
only 512KB of weights, not all 8MB). The host applies the top-2 combine
weights and scatters rows back into the full [B, S, H] output.
"""

import os
import numpy as np
import ml_dtypes

H = 1024
I = 2048
E = 8
TOPK = 2
P = 128
N_T = 3  # token chunks per core (chunk width C/3 <= 512 = one PSUM bank)

_PROGRAM_CACHE = {}
LAST_RESULT = None  # BassKernelResults of the most recent device run


def _build_program(C):
    from contextlib import ExitStack

    import concourse.mybir as mybir
    import concourse.tile as tile
    from concourse import bacc

    f32 = mybir.dt.float32
    bf16 = mybir.dt.bfloat16
    Silu = mybir.ActivationFunctionType.Silu

    n_h = H // P   # 8 contraction chunks over hidden dim
    n_i = I // P   # 16 tiles over intermediate dim
    NT = C // N_T  # token-chunk width
    assert C % N_T == 0 and NT <= 512

    nc = bacc.Bacc("TRN2")
    xT = nc.dram_tensor("xT", [H, C], bf16, kind="ExternalInput")
    # gate/up are host-prearranged to [p, i_tile, c, j] so each per-i-tile
    # DMA reads 2KB-contiguous lines per partition (full DMA rate).
    wg = nc.dram_tensor("wg", [P, I // P, H // P, P], bf16, kind="ExternalInput")
    wu = nc.dram_tensor("wu", [P, I // P, H // P, P], bf16, kind="ExternalInput")
    wd = nc.dram_tensor("wd", [I, H], bf16, kind="ExternalInput")
    yT = nc.dram_tensor("yT", [H, C], f32, kind="ExternalOutput")

    with tile.TileContext(nc) as tc:
        with ExitStack() as ctx:
            wpool = ctx.enter_context(tc.tile_pool(name="weights", bufs=1))
            atpool = ctx.enter_context(tc.tile_pool(name="atp", bufs=1))
            spool = ctx.enter_context(tc.tile_pool(name="stmp", bufs=4))
            ypool = ctx.enter_context(tc.tile_pool(name="yst", bufs=4))
            pspool = ctx.enter_context(
                tc.tile_pool(name="ps", bufs=8, space="PSUM")
            )

            x_s = wpool.tile([P, n_h, C], bf16, name="x_s")
            # gate/up keyed by i-tile: [p, i_tile, c, i_within]
            wg_s = wpool.tile([P, n_i, n_h, P], bf16, name="wg_s")
            wu_s = wpool.tile([P, n_i, n_h, P], bf16, name="wu_s")
            wd_s = wpool.tile([P, n_i, H], bf16, name="wd_s")
            at_s = atpool.tile([P, n_i, C], bf16, name="at_s")

            # Load order = consumption order. Two independent HWDGE FIFO
            # rings: x rides the ACT ring (per-chunk transfers so arrival is
            # incremental), weights ride the SP ring (per-i-tile gate/up
            # column blocks, then wd which phase B only needs ~150us in).
            # Rings drain concurrently, so x never queues behind weights.
            nc.scalar.dma_start(out=x_s[:, 0, 0:NT], in_=xT[0:P, 0:NT])
            nc.scalar.dma_start(out=x_s[:, 0, NT:C], in_=xT[0:P, NT:C])
            for c in range(1, n_h):
                nc.scalar.dma_start(
                    out=x_s[:, c, :], in_=xT[c * P:(c + 1) * P, :]
                )
            for it in range(n_i):
                nc.sync.dma_start(out=wg_s[:, it, :, :], in_=wg[:, it, :, :])
                nc.sync.dma_start(out=wu_s[:, it, :, :], in_=wu[:, it, :, :])
            for it in range(n_i):
                nc.sync.dma_start(
                    out=wd_s[:, it, :], in_=wd[it * P:(it + 1) * P, :]
                )

            # Phase A: AT[i_tile, tok] = silu(Wg^T X) * (Wu^T X)
            for it in range(n_i):
                g_ps = [
                    pspool.tile([P, NT], f32, tag="ps", name=f"g_{it}_{k}")
                    for k in range(N_T)
                ]
                u_ps = [
                    pspool.tile([P, NT], f32, tag="ps", name=f"u_{it}_{k}")
                    for k in range(N_T)
                ]
                for c in range(n_h):
                    lg = wg_s[:, it, c, :]
                    lu = wu_s[:, it, c, :]
                    st, sp = (c == 0), (c == n_h - 1)
                    for k in range(N_T):
                        nc.tensor.matmul(
                            g_ps[k], lg, x_s[:, c, k * NT:(k + 1) * NT],
                            start=st, stop=sp,
                        )
                    for k in range(N_T):
                        nc.tensor.matmul(
                            u_ps[k], lu, x_s[:, c, k * NT:(k + 1) * NT],
                            start=st, stop=sp,
                        )
                for k in range(N_T):
                    stile = spool.tile([P, NT], f32, tag="stmp", name=f"s_{it}_{k}")
                    nc.scalar.activation(stile, g_ps[k], Silu)
                    nc.vector.tensor_mul(
                        at_s[:, it, k * NT:(k + 1) * NT], stile, u_ps[k]
                    )

            # Phase B: YT[h_tile, tok] = sum_i Wd[i, h_tile]^T AT[i, tok]
            for ht in range(n_h):
                y_ps = [
                    pspool.tile([P, NT], f32, tag="ps", name=f"y_{ht}_{k}")
                    for k in range(N_T)
                ]
                for it in range(n_i):
                    ld = wd_s[:, it, ht * P:(ht + 1) * P]
                    st, sp = (it == 0), (it == n_i - 1)
                    for k in range(N_T):
                        nc.tensor.matmul(
                            y_ps[k], ld, at_s[:, it, k * NT:(k + 1) * NT],
                            start=st, stop=sp,
                        )
                for k in range(N_T):
                    yt = ypool.tile([P, NT], f32, tag="yst", name=f"yo_{ht}_{k}")
                    nc.vector.tensor_copy(yt, y_ps[k])
                    nc.scalar.dma_start(
                        out=yT[ht * P:(ht + 1) * P, k * NT:(k + 1) * NT], in_=yt
                    )

    nc.compile()
    return nc


def kernel(x, gate_w, wg, wu, wd):
    global LAST_RESULT
    x = np.asarray(x, dtype=np.float32)
    gate_w = np.asarray(gate_w, dtype=np.float32)
    wg = np.asarray(wg, dtype=np.float32)
    wu = np.asarray(wu, dtype=np.float32)
    wd = np.asarray(wd, dtype=np.float32)

    B, S, Hh = x.shape
    T = B * S
    xf = np.ascontiguousarray(x.reshape(T, Hh))

    # Router (tiny): logits -> softmax -> top-2, matching jax.lax.top_k
    # tie-order (stable sort prefers the lower expert index).
    logits = xf @ gate_w.T
    logits -= logits.max(axis=-1, keepdims=True)
    np.exp(logits, out=logits)
    probs = logits / logits.sum(axis=-1, keepdims=True)
    order = np.argsort(-probs, axis=1, kind="stable")[:, :TOPK]

    onehot = np.zeros((T, E), dtype=bool)
    onehot[np.arange(T)[:, None], order] = True
    tok_lists = [np.nonzero(onehot[:, e])[0] for e in range(E)]
    maxc = max(len(t) for t in tok_lists)
    C = int(-(-maxc // N_T) * N_T)  # round up to a multiple of N_T
    if C // N_T > 512:  # token chunk must fit one PSUM bank
        C = int(-(-maxc // P) * P)
        # fall back to wider padding so C/N_T stays <= 512 if ever needed
        while C // N_T > 512:
            C += N_T

    nc = _PROGRAM_CACHE.get(C)
    if nc is None:
        nc = _build_program(C)
        _PROGRAM_CACHE[C] = nc

    bf = ml_dtypes.bfloat16
    xf_bf = xf.astype(bf)

    def _gu_layout(w):  # [H, I] -> [p, i_tile, c, j]
        return np.ascontiguousarray(
            w.reshape(H // P, P, I // P, P).transpose(1, 2, 0, 3)
        )

    in_maps = []
    for e in range(E):
        idx = tok_lists[e]
        xe = np.zeros((C, Hh), dtype=bf)
        xe[: len(idx)] = xf_bf[idx]
        in_maps.append(
            {
                "xT": np.ascontiguousarray(xe.T),
                "wg": _gu_layout(wg[e].astype(bf)),
                "wu": _gu_layout(wu[e].astype(bf)),
                "wd": wd[e].astype(bf),
            }
        )

    from concourse.bass_utils import run_bass_kernel_spmd

    res = run_bass_kernel_spmd(nc, in_maps, core_ids=list(range(E)))
    LAST_RESULT = res

    out = np.zeros((T, Hh), dtype=np.float32)
    for e in range(E):
        idx = tok_lists[e]
        ye = np.asarray(res.results[e]["yT"]).T[: len(idx)]
        out[idx] += probs[idx, e][:, None] * ye.astype(np.float32)
    return out.reshape(B, S, Hh)


# revision 11
# speedup vs baseline: 1.0009x; 1.0009x over previous
"""MoE feed-forward (E=8 experts, top-2) for one TRN2 chip (8 NeuronCores).

Strategy: expert-parallel. Host computes the (tiny) router matmul + softmax
+ top-2 in numpy, gathers each expert's routed tokens, pads to a fixed
capacity C, and ships per-expert weights + gathered tokens to one core each.
Each core runs an identical Bass/Tile FFN program in bf16:

    GT = Wg^T @ X   (transposed-activation layout: [I, C] tiles)
    UT = Wu^T @ X
    AT = silu(GT) * UT          (bf16, SBUF-resident)
    YT = Wd^T_col-tiles @ AT    -> [H, C] f32 out

All matmul operands use natural (row-major chunk) layouts, so no on-device
transposes are needed. Gate/up weights are DMA'd in per-i-tile column blocks
so arrival order matches the PE's consumption order (the first i-tile needs
only 512KB of weights, not all 8MB). The host applies the top-2 combine
weights and scatters rows back into the full [B, S, H] output.
"""

import os
import numpy as np
import ml_dtypes

H = 1024
I = 2048
E = 8
TOPK = 2
P = 128
N_T = 3  # token chunks per core (chunk width C/3 <= 512 = one PSUM bank)

_PROGRAM_CACHE = {}
LAST_RESULT = None  # BassKernelResults of the most recent device run


def _build_program(C):
    from contextlib import ExitStack

    import concourse.mybir as mybir
    import concourse.tile as tile
    from concourse import bacc

    f32 = mybir.dt.float32
    bf16 = mybir.dt.bfloat16
    Silu = mybir.ActivationFunctionType.Silu

    n_h = H // P   # 8 contraction chunks over hidden dim
    n_i = I // P   # 16 tiles over intermediate dim
    NT = C // N_T  # token-chunk width
    assert C % N_T == 0 and NT <= 512

    nc = bacc.Bacc("TRN2", enable_partition_id=False)
    xT = nc.dram_tensor("xT", [H, C], bf16, kind="ExternalInput")
    # gate/up are host-prearranged to [p, i_tile, c, j] so each per-i-tile
    # DMA reads 2KB-contiguous lines per partition (full DMA rate).
    wg = nc.dram_tensor("wg", [P, I // P, H // P, P], bf16, kind="ExternalInput")
    wu = nc.dram_tensor("wu", [P, I // P, H // P, P], bf16, kind="ExternalInput")
    wd = nc.dram_tensor("wd", [I, H], bf16, kind="ExternalInput")
    yT = nc.dram_tensor("yT", [H, C], f32, kind="ExternalOutput")

    with tile.TileContext(nc) as tc:
        with ExitStack() as ctx:
            wpool = ctx.enter_context(tc.tile_pool(name="weights", bufs=1))
            atpool = ctx.enter_context(tc.tile_pool(name="atp", bufs=1))
            spool = ctx.enter_context(tc.tile_pool(name="stmp", bufs=4))
            ypool = ctx.enter_context(tc.tile_pool(name="yst", bufs=4))
            pspool = ctx.enter_context(
                tc.tile_pool(name="ps", bufs=8, space="PSUM")
            )

            x_s = wpool.tile([P, n_h, C], bf16, name="x_s")
            # gate/up keyed by i-tile: [p, i_tile, c, i_within]
            wg_s = wpool.tile([P, n_i, n_h, P], bf16, name="wg_s")
            wu_s = wpool.tile([P, n_i, n_h, P], bf16, name="wu_s")
            wd_s = wpool.tile([P, n_i, H], bf16, name="wd_s")
            at_s = atpool.tile([P, n_i, C], bf16, name="at_s")

            # Load order = consumption order, all on the SP HWDGE ring whose
            # FIFO order IS the priority order: the first token-chunk of x
            # chunk 0 gates the very first matmul; gate/up for i-tile 0; the
            # rest of x (needed within i-tile 0's contraction sweep);
            # remaining gate/up blocks just-in-time; then wd (phase B only
            # needs it ~140us in).
            nc.sync.dma_start(out=x_s[:, 0, 0:NT], in_=xT[0:P, 0:NT])
            nc.sync.dma_start(out=wg_s[:, 0, :, :], in_=wg[:, 0, :, :])
            nc.sync.dma_start(out=wu_s[:, 0, :, :], in_=wu[:, 0, :, :])
            nc.sync.dma_start(out=x_s[:, 0, NT:C], in_=xT[0:P, NT:C])
            for c in range(1, n_h):
                nc.sync.dma_start(
                    out=x_s[:, c, :], in_=xT[c * P:(c + 1) * P, :]
                )
            for it in range(1, n_i):
                nc.sync.dma_start(out=wg_s[:, it, :, :], in_=wg[:, it, :, :])
                nc.sync.dma_start(out=wu_s[:, it, :, :], in_=wu[:, it, :, :])
            for it in range(n_i):
                nc.sync.dma_start(
                    out=wd_s[:, it, :], in_=wd[it * P:(it + 1) * P, :]
                )

            # Phase A: AT[i_tile, tok] = silu(Wg^T X) * (Wu^T X)
            for it in range(n_i):
                g_ps = [
                    pspool.tile([P, NT], f32, tag="ps", name=f"g_{it}_{k}")
                    for k in range(N_T)
                ]
                u_ps = [
                    pspool.tile([P, NT], f32, tag="ps", name=f"u_{it}_{k}")
                    for k in range(N_T)
                ]
                for c in range(n_h):
                    lg = wg_s[:, it, c, :]
                    lu = wu_s[:, it, c, :]
                    st, sp = (c == 0), (c == n_h - 1)
                    for k in range(N_T):
                        nc.tensor.matmul(
                            g_ps[k], lg, x_s[:, c, k * NT:(k + 1) * NT],
                            start=st, stop=sp,
                        )
                    for k in range(N_T):
                        nc.tensor.matmul(
                            u_ps[k], lu, x_s[:, c, k * NT:(k + 1) * NT],
                            start=st, stop=sp,
                        )
                for k in range(N_T):
                    stile = spool.tile([P, NT], f32, tag="stmp", name=f"s_{it}_{k}")
                    nc.scalar.activation(stile, g_ps[k], Silu)
                    nc.vector.tensor_mul(
                        at_s[:, it, k * NT:(k + 1) * NT], stile, u_ps[k]
                    )

            # Phase B: YT[h_tile, tok] = sum_i Wd[i, h_tile]^T AT[i, tok]
            for ht in range(n_h):
                y_ps = [
                    pspool.tile([P, NT], f32, tag="ps", name=f"y_{ht}_{k}")
                    for k in range(N_T)
                ]
                for it in range(n_i):
                    ld = wd_s[:, it, ht * P:(ht + 1) * P]
                    st, sp = (it == 0), (it == n_i - 1)
                    for k in range(N_T):
                        nc.tensor.matmul(
                            y_ps[k], ld, at_s[:, it, k * NT:(k + 1) * NT],
                            start=st, stop=sp,
                        )
                for k in range(N_T):
                    yt = ypool.tile([P, NT], f32, tag="yst", name=f"yo_{ht}_{k}")
                    nc.vector.tensor_copy(yt, y_ps[k])
                    nc.gpsimd.dma_start(
                        out=yT[ht * P:(ht + 1) * P, k * NT:(k + 1) * NT], in_=yt
                    )

    nc.compile()
    return nc


def kernel(x, gate_w, wg, wu, wd):
    global LAST_RESULT
    x = np.asarray(x, dtype=np.float32)
    gate_w = np.asarray(gate_w, dtype=np.float32)
    wg = np.asarray(wg, dtype=np.float32)
    wu = np.asarray(wu, dtype=np.float32)
    wd = np.asarray(wd, dtype=np.float32)

    B, S, Hh = x.shape
    T = B * S
    xf = np.ascontiguousarray(x.reshape(T, Hh))

    # Router (tiny): logits -> softmax -> top-2, matching jax.lax.top_k
    # tie-order (stable sort prefers the lower expert index).
    logits = xf @ gate_w.T
    logits -= logits.max(axis=-1, keepdims=True)
    np.exp(logits, out=logits)
    probs = logits / logits.sum(axis=-1, keepdims=True)
    order = np.argsort(-probs, axis=1, kind="stable")[:, :TOPK]

    onehot = np.zeros((T, E), dtype=bool)
    onehot[np.arange(T)[:, None], order] = True
    tok_lists = [np.nonzero(onehot[:, e])[0] for e in range(E)]
    maxc = max(len(t) for t in tok_lists)
    C = int(-(-maxc // N_T) * N_T)  # round up to a multiple of N_T
    if C // N_T > 512:  # token chunk must fit one PSUM bank
        C = int(-(-maxc // P) * P)
        # fall back to wider padding so C/N_T stays <= 512 if ever needed
        while C // N_T > 512:
            C += N_T

    nc = _PROGRAM_CACHE.get(C)
    if nc is None:
        nc = _build_program(C)
        _PROGRAM_CACHE[C] = nc

    bf = ml_dtypes.bfloat16
    xf_bf = xf.astype(bf)

    def _gu_layout(w):  # [H, I] -> [p, i_tile, c, j]
        return np.ascontiguousarray(
            w.reshape(H // P, P, I // P, P).transpose(1, 2, 0, 3)
        )

    in_maps = []
    for e in range(E):
        idx = tok_lists[e]
        xe = np.zeros((C, Hh), dtype=bf)
        xe[: len(idx)] = xf_bf[idx]
        in_maps.append(
            {
                "xT": np.ascontiguousarray(xe.T),
                "wg": _gu_layout(wg[e].astype(bf)),
                "wu": _gu_layout(wu[e].astype(bf)),
                "wd": wd[e].astype(bf),
            }
        )

    from concourse.bass_utils import run_bass_kernel_spmd

    res = run_bass_kernel_spmd(nc, in_maps, core_ids=list(range(E)))
    LAST_RESULT = res

    out = np.zeros((T, Hh), dtype=np.float32)
    for e in range(E):
        idx = tok_lists[e]
        ye = np.asarray(res.results[e]["yT"]).T[: len(idx)]
        out[idx] += probs[idx, e][:, None] * ye.astype(np.float32)
    return out.reshape(B, S, Hh)
